# revision 15
# baseline (speedup 1.0000x reference)
"""Bidirectional Mamba (4 layers x 2 dirs, B=4, L=1024, d_model=768,
d_inner=1536, d_state=16) on 8 TRN2 NeuronCores.

Sharding: core c handles (batch b = c%4, direction d = c//4). Each core runs
the full 4-layer stack for its (b, dir) stream in feature-major layout
[channels-on-partitions, tokens-on-free]. The per-layer h = out_fwd + out_rev
exchange is a pair AllReduce over cores {c, c+4}. The kernel program is
identical on all cores (SPMD); the direction flip is handled by per-core 0/1
select masks applied to (h, flip(h)), and all other per-dir differences are
input data.

Key mappings:
 - depthwise causal conv folded into the in_proj matmul on PE:
   host precomputes W_k[d,e] = conv_w[d,k] * in_w_x[d,e]; the kernel runs 4
   accumulated matmuls against token-shifted views of a zero-left-padded hn.
 - selective scan: p = sigmoid(-(dt_raw+dtb)) so logp = -delta; per state i,
   dA_i = exp(apos_i * logp) on ACT (apos = exp(A_log) per-partition scale),
   dB_i = w * (-B_i) with w = logp*u (so w = -delta*u), then
   tensor_tensor_scan on DVE, prod = s*C_i, accumulated over i.
 - LN in feature-major via ones-matmul column sums on PE.
"""
import os
import numpy as np
import ml_dtypes
from contextlib import ExitStack

import concourse.bacc as bacc
import concourse.tile as tile
import concourse.mybir as mybir
from concourse.bass_utils import run_bass_kernel_spmd

F32 = mybir.dt.float32
BF16 = mybir.dt.bfloat16
MUL = mybir.AluOpType.mult
ADD = mybir.AluOpType.add
SUB = mybir.AluOpType.subtract
AF = mybir.ActivationFunctionType

P = 128
B, L, D, DI, DS, DR, DC, NL = 4, 1024, 768, 1536, 16, 48, 4, 4
EPS = 1e-5
KF = D // P        # 6  k-folds of d_model
MF = DI // P       # 12 folds of d_inner
NT = L // 512      # 2  N-tiles per token row
CH = 2             # folds per scan chunk
NCH = MF // CH     # 6 chunks
MARG = DC - 1      # 3 left-pad columns for causal conv

_CACHE = {}


def _build():
    KNL = int(os.environ.get("K_NL", NL))
    KPH = int(os.environ.get("K_PH", 9))
    nc = bacc.Bacc("TRN2", target_bir_lowering=False)

    # ---------------- dram I/O ----------------
    x0T_d = nc.dram_tensor("x0T", [D, L], F32, kind="ExternalInput")
    lnw_d = nc.dram_tensor("lnw", [NL, P, KF], F32, kind="ExternalInput")
    lnb_d = nc.dram_tensor("lnb", [NL, P, KF], F32, kind="ExternalInput")
    inwT_d = nc.dram_tensor("inwT", [NL, D, 5 * DI], BF16, kind="ExternalInput")
    cb_d = nc.dram_tensor("cb", [NL, P, MF], F32, kind="ExternalInput")
    xpwT_d = nc.dram_tensor("xpwT", [NL, DI, 96], BF16, kind="ExternalInput")
    dtwT_d = nc.dram_tensor("dtwT", [NL, DR, DI], BF16, kind="ExternalInput")
    dtbn_d = nc.dram_tensor("dtbn", [NL, P, MF], F32, kind="ExternalInput")
    apos_d = nc.dram_tensor("apos", [NL, P, DS], F32, kind="ExternalInput")
    dsk_d = nc.dram_tensor("dsk", [NL, P, MF], F32, kind="ExternalInput")
    owT_d = nc.dram_tensor("owT", [NL, DI, D], BF16, kind="ExternalInput")
    fw_d = nc.dram_tensor("fw", [P, KF], F32, kind="ExternalInput")
    fb_d = nc.dram_tensor("fb", [P, KF], F32, kind="ExternalInput")
    sel_d = nc.dram_tensor("sel", [P, 2], F32, kind="ExternalInput")
    o_d = nc.dram_tensor("o_fm", [D, L], F32, kind="ExternalOutput")

    z_dram = nc.dram_tensor("z_sp", [MF, P, L], BF16)
    u_dram = nc.dram_tensor("u_sp", [MF, P, L], BF16)
    y_dram = nc.dram_tensor("y_sp", [MF, P, L], BF16)
    g_dram = nc.dram_tensor("g_sp", [MF, P, L], BF16)
    bc_dram = nc.dram_tensor("bc_sp", [2 * DS, L], BF16)
    cc_in = [nc.dram_tensor(f"cc_in{j}", [D, L], F32) for j in range(NL)]
    cc_out = [nc.dram_tensor(f"cc_out{j}", [D, L], F32) for j in range(NL)]

    with tile.TileContext(nc) as tc, ExitStack() as ctx:
        pers = ctx.enter_context(tc.tile_pool(name="pers", bufs=1))
        vpool = ctx.enter_context(tc.tile_pool(name="vpool", bufs=2))
        big = ctx.enter_context(tc.tile_pool(name="big", bufs=1))
        stg = ctx.enter_context(tc.tile_pool(name="stg", bufs=5))
        stg32 = ctx.enter_context(tc.tile_pool(name="stg32", bufs=4))
        wbig = ctx.enter_context(tc.tile_pool(name="wbig", bufs=3))
        prm = ctx.enter_context(tc.tile_pool(name="prm", bufs=2))
        dApool = ctx.enter_context(tc.tile_pool(name="dApool", bufs=2))
        dBpool = ctx.enter_context(tc.tile_pool(name="dBpool", bufs=3))
        spool = ctx.enter_context(tc.tile_pool(name="spool", bufs=2))
        accp = ctx.enter_context(tc.tile_pool(name="accp", bufs=3))
        bcp = ctx.enter_context(tc.tile_pool(name="bcp", bufs=4))
        mm = ctx.enter_context(tc.tile_pool(name="mm", bufs=8, space="PSUM"))

        # persistent across layers
        h32 = pers.tile([P, KF, L], F32, name="h32")
        res16 = vpool.tile([P, KF, L], BF16, name="res16", tag="resp")
        ones16 = pers.tile([P, 1], BF16, name="ones16")
        sel_sb = pers.tile([P, 2], F32, name="sel_sb")
        nc.vector.memset(res16[:], 0.0)
        nc.vector.memset(ones16[:], 1.0)
        nc.sync.dma_start(sel_sb[:], sel_d[:])
        nc.sync.dma_start(h32[:], x0T_d[:].rearrange("(f p) l -> p f l", p=P))

        def ln_feature_major(vin, wcol, bcol, hn_out, out_off, fp32_rows):
            """LN over the 768 partition-channels of vin [P,KF,L] (bf16).
            hn_out[:, f, out_off:out_off+L] gets normalized*w+b output."""
            ps_s = [mm.tile([P, 512], F32, name="lnps", tag="ps") for _ in range(2 * NT)]
            for f in range(KF):
                sq = stg.tile([P, L], BF16, name="stg_a", tag="st16")
                nc.scalar.activation(sq[:], vin[:, f, :], AF.Square)
                for n in range(NT):
                    nc.tensor.matmul(
                        ps_s[n][0:1, :], ones16[:], vin[:, f, n * 512:(n + 1) * 512],
                        start=(f == 0), stop=(f == KF - 1))
                    nc.tensor.matmul(
                        ps_s[NT + n][0:1, :], ones16[:], sq[:, n * 512:(n + 1) * 512],
                        start=(f == 0), stop=(f == KF - 1))
            mu_r = stg32.tile([1, L], F32, name="mu_r", tag="st32")
            for n in range(NT):
                nc.vector.tensor_scalar(
                    out=mu_r[:, n * 512:(n + 1) * 512], in0=ps_s[n][0:1, :],
                    scalar1=1.0 / D, scalar2=None, op0=MUL)
            mu2_r = stg32.tile([1, L], F32, name="mu2_r", tag="st32")
            nc.vector.tensor_tensor(out=mu2_r[:], in0=mu_r[:], in1=mu_r[:], op=MUL)
            var_r = stg32.tile([1, L], F32, name="var_r", tag="st32")
            for n in range(NT):
                nc.vector.scalar_tensor_tensor(
                    out=var_r[:, n * 512:(n + 1) * 512], in0=ps_s[NT + n][0:1, :],
                    scalar=1.0 / D, in1=mu2_r[:, n * 512:(n + 1) * 512],
                    op0=MUL, op1=SUB)
            eps_r = stg32.tile([1, L], F32, name="eps_r", tag="st32")
            nc.vector.memset(eps_r[:], EPS)
            sd_r = stg32.tile([1, L], F32, name="sd_r", tag="st32")
            nc.scalar.activation(sd_r[:], var_r[:], AF.Sqrt, bias=eps_r[:, 0:1])
            rstd_r = stg32.tile([1, L], F32, name="rstd_r", tag="st32")
            nc.vector.reciprocal(rstd_r[:], sd_r[:])
            dt_b = F32 if fp32_rows else BF16
            mu_b = (stg32 if fp32_rows else bcp).tile([P, L], dt_b, name="mu_b", tag="st32" if fp32_rows else "nbcb")
            rstd_b = (stg32 if fp32_rows else bcp).tile([P, L], dt_b, name="rstd_b", tag="st32" if fp32_rows else "nbcb")
            if fp32_rows:
                nc.gpsimd.partition_broadcast(mu_b[:], mu_r[:])
                nc.gpsimd.partition_broadcast(rstd_b[:], rstd_r[:])
            else:
                mu16_r = stg.tile([1, L], BF16, name="mu16_r", tag="st16")
                rstd16_r = stg.tile([1, L], BF16, name="rstd16_r", tag="st16")
                nc.vector.tensor_copy(mu16_r[:], mu_r[:])
                nc.vector.tensor_copy(rstd16_r[:], rstd_r[:])
                nc.gpsimd.partition_broadcast(mu_b[:], mu16_r[:])
                nc.gpsimd.partition_broadcast(rstd_b[:], rstd16_r[:])
            for f in range(KF):
                st1 = stg.tile([P, L], dt_b, name="stg_b", tag="st16")
                nc.vector.tensor_tensor(out=st1[:], in0=vin[:, f, :], in1=mu_b[:], op=SUB)
                st2 = stg.tile([P, L], dt_b, name="stg_c", tag="st16")
                nc.vector.tensor_tensor(out=st2[:], in0=st1[:], in1=rstd_b[:], op=MUL)
                nc.vector.scalar_tensor_tensor(
                    out=hn_out[:, f, out_off:out_off + L], in0=st2[:],
                    scalar=wcol[:, f:f + 1],
                    in1=bcol[:, f:f + 1].to_broadcast([P, L]),
                    op0=MUL, op1=ADD)

        for j in range(KNL):
            # ---- per-layer params ----
            lnw = prm.tile([P, KF], F32, name="lnw")
            lnb = prm.tile([P, KF], F32, name="lnb")
            cbt = prm.tile([P, MF], F32, name="cbt")
            dtbn = prm.tile([P, MF], F32, name="dtbn")
            apos = prm.tile([P, DS], F32, name="apos")
            dsk = prm.tile([P, MF], F32, name="dsk")
            nc.sync.dma_start(lnw[:], lnw_d[j])
            nc.sync.dma_start(lnb[:], lnb_d[j])
            nc.sync.dma_start(cbt[:], cb_d[j])
            nc.sync.dma_start(dtbn[:], dtbn_d[j])
            nc.sync.dma_start(apos[:], apos_d[j])
            nc.sync.dma_start(dsk[:], dsk_d[j])

            # ---- v = sel0*h + sel1*flip(h) + res ; res' = h + flip(h) + 2res
            v16a = vpool.tile([P, KF, L], BF16, name="v16", tag="vp")
            nc.vector.scalar_tensor_tensor(
                out=v16a[:], in0=h32[:], scalar=sel_sb[:, 0:1], in1=res16[:],
                op0=MUL, op1=ADD)
            v16 = vpool.tile([P, KF, L], BF16, name="v16b", tag="vp")
            nc.vector.scalar_tensor_tensor(
                out=v16[:], in0=h32[:, :, ::-1], scalar=sel_sb[:, 1:2], in1=v16a[:],
                op0=MUL, op1=ADD)
            tmp16 = vpool.tile([P, KF, L], BF16, name="tmp16", tag="vp")
            nc.vector.tensor_tensor(out=tmp16[:], in0=h32[:], in1=h32[:, :, ::-1], op=ADD)
            res_new = vpool.tile([P, KF, L], BF16, name="res_new", tag="resp")
            nc.vector.scalar_tensor_tensor(
                out=res_new[:], in0=res16[:], scalar=2.0, in1=tmp16[:], op0=MUL, op1=ADD)
            res16 = res_new

            # ---- LN -> hn (left-padded by MARG zero cols) ----
            hn16 = big.tile([P, KF, MARG + L], BF16, name="hn16", tag="bigA")
            nc.vector.memset(hn16[:, :, 0:MARG], 0.0)
            ln_feature_major(v16, lnw, lnb, hn16, MARG, fp32_rows=False)

            if KPH < 1:
                continue
            # ---- in_proj x-half with folded conv -> silu -> u ----
            for m in range(MF):
                ps = [mm.tile([P, 512], F32, name="ps_ip", tag="ps") for _ in range(NT)]
                for k in range(KF):
                    wk = wbig.tile([P, 4 * P], BF16, name="wk_ip", tag="w")
                    # lhsT slab: rows k-fold, cols = 4 taps x this m tile
                    nc.sync.dma_start(
                        wk[:].rearrange("p (t q) -> p t q", t=4),
                        inwT_d[j, k * P:(k + 1) * P, :]
                        .rearrange("p (t di) -> p t di", t=5)[:, 0:4, m * P:(m + 1) * P])
                    for tap in range(DC):
                        for n in range(NT):
                            nc.tensor.matmul(
                                ps[n], wk[:, tap * P:(tap + 1) * P],
                                hn16[:, k, tap + n * 512: tap + n * 512 + 512],
                                start=(k == 0 and tap == 0),
                                stop=(k == KF - 1 and tap == DC - 1))
                u_st = stg.tile([P, L], BF16, name="stg_u", tag="st16")
                for n in range(NT):
                    nc.scalar.activation(
                        u_st[:, n * 512:(n + 1) * 512], ps[n], AF.Silu,
                        bias=cbt[:, m:m + 1])
                nc.sync.dma_start(u_dram[m], u_st[:])

            if KPH < 2:
                continue
            # ---- in_proj z-half -> z_dram ----
            for m in range(MF):
                ps = [mm.tile([P, 512], F32, name="ps_ip", tag="ps") for _ in range(NT)]
                for k in range(KF):
                    wz = wbig.tile([P, P], BF16, name="wz_ip", tag="w")
                    nc.sync.dma_start(
                        wz[:], inwT_d[j, k * P:(k + 1) * P,
                                      4 * DI + m * P:4 * DI + (m + 1) * P])
                    for n in range(NT):
                        nc.tensor.matmul(
                            ps[n], wz[:],
                            hn16[:, k, MARG + n * 512: MARG + n * 512 + 512],
                            start=(k == 0), stop=(k == KF - 1))
                z_st = stg.tile([P, L], BF16, name="stg_z", tag="st16")
                for n in range(NT):
                    eng = nc.vector if n == 0 else nc.scalar
                    if n == 0:
                        nc.vector.tensor_copy(z_st[:, 0:512], ps[0])
                    else:
                        nc.scalar.copy(z_st[:, 512:1024], ps[1])
                nc.sync.dma_start(z_dram[m], z_st[:])

            if KPH < 3:
                continue
            # ---- x_proj ----
            ps_xd = [mm.tile([P, 512], F32, name="ps_xd", tag="ps") for _ in range(NT)]
            for k in range(MF):
                xw = wbig.tile([P, 96], BF16, name="xw_xp", tag="w")
                nc.sync.dma_start(xw[:], xpwT_d[j, k * P:(k + 1) * P, :])
                u_rd = stg.tile([P, L], BF16, name="stg_u2", tag="st16")
                nc.sync.dma_start(u_rd[:], u_dram[k])
                for n in range(NT):
                    nc.tensor.matmul(
                        ps_xd[n][0:96, :], xw[:], u_rd[:, n * 512:(n + 1) * 512],
                        start=(k == 0), stop=(k == MF - 1))
            xd16 = prm.tile([96, L], BF16, name="xd16")
            for n in range(NT):
                sl = slice(n * 512, (n + 1) * 512)
                nc.vector.tensor_copy(xd16[0:DR, sl], ps_xd[n][0:DR, :])
                nc.scalar.copy(xd16[64:96, sl], ps_xd[n][64:96, :])
            nc.sync.dma_start(bc_dram[:], xd16[64:96, :])

            if KPH < 4:
                continue
            # ---- dt_proj ----
            logp16 = big.tile([P, MF, L], BF16, name="logp16", tag="bigB")
            for m in range(MF):
                ps = [mm.tile([P, 512], F32, name="ps_dt", tag="ps") for _ in range(NT)]
                dw = wbig.tile([DR, P], BF16, name="dw_dt", tag="w")
                nc.sync.dma_start(dw[:], dtwT_d[j, :, m * P:(m + 1) * P])
                for n in range(NT):
                    nc.tensor.matmul(ps[n], dw[:], xd16[0:DR, n * 512:(n + 1) * 512],
                                     start=True, stop=True)
                for n in range(NT):
                    p_st = stg.tile([P, 512], BF16, name="stg_p", tag="st16")
                    nc.scalar.activation(p_st[:], ps[n], AF.Sigmoid,
                                         bias=dtbn[:, m:m + 1], scale=-1.0)
                    nc.scalar.activation(logp16[:, m, n * 512:(n + 1) * 512],
                                         p_st[:], AF.Ln)

            if KPH < 5:
                continue
            # ---- w = logp*u ----
            w16 = big.tile([P, MF, L], BF16, name="w16", tag="bigA")
            for k in range(MF):
                u_rd = stg.tile([P, L], BF16, name="stg_u3", tag="st16")
                nc.sync.dma_start(u_rd[:], u_dram[k])
                nc.vector.scalar_tensor_tensor(
                    out=w16[:, k, :], in0=logp16[:, k, :], scalar=-1.0,
                    in1=u_rd[:], op0=MUL, op1=MUL)
            # poison t=0 so fold-chained scans reset state at fold boundaries
            nc.vector.memset(logp16[:, :, 0:1], -30000.0)

            if KPH < 6:
                continue
            # ---- scan ----
            for ch in range(NCH):
                fs = slice(ch * CH, (ch + 1) * CH)
                accE = None
                accO = None
                for i in range(DS):
                    nb = bcp.tile([P, L], BF16, name="nb_bc", tag="nbcb")
                    nc.sync.dma_start(nb[:], bc_dram[i:i + 1, :].to_broadcast([P, L]))
                    cb_i = bcp.tile([P, L], BF16, name="cb_bc", tag="nbcb")
                    nc.sync.dma_start(cb_i[:], bc_dram[DS + i:DS + i + 1, :]
                                      .to_broadcast([P, L]))
                    dA = dApool.tile([P, CH, L], BF16, name="dA")
                    nc.scalar.activation(
                        dA[:].rearrange("p a b -> p (a b)"),
                        logp16[:, fs, :].rearrange("p a b -> p (a b)"),
                        AF.Exp, scale=apos[:, i:i + 1])
                    dB = dBpool.tile([P, CH, L], BF16, name="dB", tag="dBp")
                    nc.vector.tensor_tensor(
                        out=dB[:], in0=w16[:, fs, :],
                        in1=nb[:, None, :].to_broadcast([P, CH, L]), op=MUL)
                    s16 = spool.tile([P, CH, L], BF16, name="s16")
                    nc.vector.tensor_tensor_scan(
                        s16[:].rearrange("p a b -> p (a b)"),
                        dA[:].rearrange("p a b -> p (a b)"),
                        dB[:].rearrange("p a b -> p (a b)"),
                        0.0, MUL, ADD)
                    prod = dBpool.tile([P, CH, L], BF16, name="prod", tag="dBp")
                    nc.vector.tensor_tensor(
                        out=prod[:], in0=s16[:],
                        in1=cb_i[:, None, :].to_broadcast([P, CH, L]), op=MUL)
                    if i < 2:
                        # initialize acc chains with u*D_skip + prod
                        tgt = accp.tile([P, CH, L], BF16, name="acc")
                        if i == 0:
                            for fo in range(CH):
                                u_rd = stg.tile([P, L], BF16, name="stg_u4", tag="st16")
                                nc.sync.dma_start(u_rd[:], u_dram[ch * CH + fo])
                                nc.vector.scalar_tensor_tensor(
                                    out=tgt[:, fo, :], in0=u_rd[:],
                                    scalar=dsk[:, ch * CH + fo:ch * CH + fo + 1],
                                    in1=prod[:, fo, :], op0=MUL, op1=ADD)
                            accE = tgt
                        else:
                            nc.vector.tensor_copy(tgt[:], prod[:])
                            accO = tgt
                    else:
                        src = accE if (i % 2 == 0) else accO
                        tgt = accp.tile([P, CH, L], BF16, name="acc")
                        nc.vector.tensor_tensor(out=tgt[:], in0=src[:], in1=prod[:], op=ADD)
                        if i % 2 == 0:
                            accE = tgt
                        else:
                            accO = tgt
                yroot = spool.tile([P, CH, L], BF16, name="s16")
                nc.vector.tensor_tensor(out=yroot[:], in0=accE[:], in1=accO[:], op=ADD)
                nc.sync.dma_start(
                    y_dram[fs].rearrange("f p t -> p f t"), yroot[:])

            if KPH < 7:
                continue
            # ---- gate ----
            for f in range(MF):
                y_st = stg.tile([P, L], BF16, name="stg_y", tag="st16")
                nc.sync.dma_start(y_st[:], y_dram[f])
                z_st = stg.tile([P, L], BF16, name="stg_z2", tag="st16")
                nc.sync.dma_start(z_st[:], z_dram[f])
                zs = stg.tile([P, L], BF16, name="stg_zs", tag="st16")
                nc.scalar.activation(zs[:], z_st[:], AF.Silu)
                g_st = stg.tile([P, L], BF16, name="stg_g", tag="st16")
                nc.vector.tensor_tensor(out=g_st[:], in0=y_st[:], in1=zs[:], op=MUL)
                nc.sync.dma_start(g_dram[f], g_st[:])

            if KPH < 8:
                continue
            # ---- out_proj ----
            for half in range(2):
                ms = range(half * 3, half * 3 + 3)
                ps_o = {(m, n): mm.tile([P, 512], F32, name="ps_op", tag="ps")
                        for m in ms for n in range(NT)}
                for k in range(MF):
                    ow = wbig.tile([P, D], BF16, name="ow_op", tag="w")
                    nc.sync.dma_start(ow[:], owT_d[j, k * P:(k + 1) * P, :])
                    g_rd = stg.tile([P, L], BF16, name="stg_g2", tag="st16")
                    nc.sync.dma_start(g_rd[:], g_dram[k])
                    for m in ms:
                        for n in range(NT):
                            nc.tensor.matmul(
                                ps_o[(m, n)], ow[:, m * P:(m + 1) * P],
                                g_rd[:, n * 512:(n + 1) * 512],
                                start=(k == 0), stop=(k == MF - 1))
                for m in ms:
                    for n in range(NT):
                        o_ev = stg32.tile([P, 512], F32, name="stg_ev", tag="st32")
                        if (m + n) % 2 == 0:
                            nc.vector.tensor_copy(o_ev[:], ps_o[(m, n)])
                        else:
                            nc.scalar.copy(o_ev[:], ps_o[(m, n)])
                        nc.sync.dma_start(
                            cc_in[j][m * P:(m + 1) * P, n * 512:(n + 1) * 512],
                            o_ev[:])

            nc.gpsimd.collective_compute(
                kind="AllReduce", op=ADD,
                replica_groups=[[0, 4], [1, 5], [2, 6], [3, 7]],
                ins=[cc_in[j][:]], outs=[cc_out[j][:]])
            h_new = pers.tile([P, KF, L], F32, name="h32", tag="h32")
            nc.sync.dma_start(h_new[:], cc_out[j][:].rearrange("(f p) l -> p f l", p=P))
            h32 = h_new

        # ---- final: out = LN(h + res) ----
        vf32 = big.tile([P, KF, L], F32, name="vf32", tag="bigA")
        nc.vector.tensor_tensor(out=vf32[:], in0=h32[:], in1=res16[:], op=ADD)
        vf16 = vpool.tile([P, KF, L], BF16, name="vf16", tag="vp")
        nc.vector.tensor_copy(vf16[:], vf32[:])
        fw = prm.tile([P, KF], F32, name="fw")
        fb = prm.tile([P, KF], F32, name="fb")
        nc.sync.dma_start(fw[:], fw_d[:])
        nc.sync.dma_start(fb[:], fb_d[:])
        # fp32 stats+normalize for the final output
        ps_s = [mm.tile([P, 512], F32, name="lnps2", tag="ps") for _ in range(2 * NT)]
        for f in range(KF):
            sq = stg.tile([P, L], BF16, name="stg_q", tag="st16")
            nc.scalar.activation(sq[:], vf16[:, f, :], AF.Square)
            for n in range(NT):
                nc.tensor.matmul(ps_s[n][0:1, :], ones16[:],
                                 vf16[:, f, n * 512:(n + 1) * 512],
                                 start=(f == 0), stop=(f == KF - 1))
                nc.tensor.matmul(ps_s[NT + n][0:1, :], ones16[:],
                                 sq[:, n * 512:(n + 1) * 512],
                                 start=(f == 0), stop=(f == KF - 1))
        mu_r = stg32.tile([1, L], F32, name="mu_rf", tag="st32")
        for n in range(NT):
            nc.vector.tensor_scalar(out=mu_r[:, n * 512:(n + 1) * 512],
                                    in0=ps_s[n][0:1, :], scalar1=1.0 / D,
                                    scalar2=None, op0=MUL)
        mu2_r = stg32.tile([1, L], F32, name="mu2_rf", tag="st32")
        nc.vector.tensor_tensor(out=mu2_r[:], in0=mu_r[:], in1=mu_r[:], op=MUL)
        var_r = stg32.tile([1, L], F32, name="var_rf", tag="st32")
        for n in range(NT):
            nc.vector.scalar_tensor_tensor(
                out=var_r[:, n * 512:(n + 1) * 512], in0=ps_s[NT + n][0:1, :],
                scalar=1.0 / D, in1=mu2_r[:, n * 512:(n + 1) * 512], op0=MUL, op1=SUB)
        eps_rf = stg32.tile([1, L], F32, name="eps_rf", tag="st32")
        nc.vector.memset(eps_rf[:], EPS)
        sd_rf = stg32.tile([1, L], F32, name="sd_rf", tag="st32")
        nc.scalar.activation(sd_rf[:], var_r[:], AF.Sqrt, bias=eps_rf[:, 0:1])
        rstd_r = stg32.tile([1, L], F32, name="rstd_rf", tag="st32")
        nc.vector.reciprocal(rstd_r[:], sd_rf[:])
        mu_b = stg32.tile([P, L], F32, name="mu_bf", tag="st32")
        rstd_b = stg32.tile([P, L], F32, name="rstd_bf", tag="st32")
        nc.gpsimd.partition_broadcast(mu_b[:], mu_r[:])
        nc.gpsimd.partition_broadcast(rstd_b[:], rstd_r[:])
        for f in range(KF):
            st1 = stg32.tile([P, L], F32, name="stg_f1", tag="st32")
            nc.vector.tensor_tensor(out=st1[:], in0=vf32[:, f, :], in1=mu_b[:], op=SUB)
            st2 = stg32.tile([P, L], F32, name="stg_f2", tag="st32")
            nc.vector.tensor_tensor(out=st2[:], in0=st1[:], in1=rstd_b[:], op=MUL)
            o_st = stg32.tile([P, L], F32, name="stg_f3", tag="st32")
            nc.vector.scalar_tensor_tensor(
                out=o_st[:], in0=st2[:], scalar=fw[:, f:f + 1],
                in1=fb[:, f:f + 1].to_broadcast([P, L]), op0=MUL, op1=ADD)
            nc.sync.dma_start(o_d[f * P:(f + 1) * P, :], o_st[:])

    nc.compile()
    return nc


def _fold(x):
    """[C] -> [P, C/P] fold-major (channel c = fold*128 + p)."""
    x = np.asarray(x, np.float32)
    nf = x.shape[-1] // P
    return np.ascontiguousarray(x.reshape(nf, P).T)


def _prep_core_inputs(inputs, b, d):
    bf = lambda x: np.ascontiguousarray(np.asarray(x)).astype(ml_dtypes.bfloat16)
    f32 = lambda x: np.ascontiguousarray(np.asarray(x, np.float32))
    inp = {k: np.asarray(v) for k, v in inputs.items()}

    inwT = np.empty((NL, D, 5 * DI), np.float32)
    lnw = np.empty((NL, P, KF), np.float32)
    lnb = np.empty((NL, P, KF), np.float32)
    cb = np.empty((NL, P, MF), np.float32)
    xpwT = np.zeros((NL, DI, 96), np.float32)
    dtwT = np.empty((NL, DR, DI), np.float32)
    dtbn = np.empty((NL, P, MF), np.float32)
    apos = np.empty((NL, P, DS), np.float32)
    dsk = np.empty((NL, P, MF), np.float32)
    owT = np.empty((NL, DI, D), np.float32)
    for j in range(NL):
        iw = np.asarray(inp["in_proj_w"][j, d], np.float32)   # (3072, 768)
        cw = np.asarray(inp["conv_w"][j, d], np.float32)      # (1536, 4)
        wx, wz = iw[:DI], iw[DI:]
        parts = [ (wx * cw[:, k:k + 1]).T for k in range(DC) ] + [wz.T]
        inwT[j] = np.concatenate(parts, axis=1)
        lnw[j] = _fold(inp["norm_w"][j, d])
        lnb[j] = _fold(inp["norm_b"][j, d])
        cb[j] = _fold(inp["conv_b"][j, d])
        xpw_t = np.asarray(inp["x_proj_w"][j, d], np.float32).T
        xpwT[j, :, 0:DR] = xpw_t[:, 0:DR]
        xpwT[j, :, 64:80] = xpw_t[:, DR:DR + DS]
        xpwT[j, :, 80:96] = xpw_t[:, DR + DS:80]
        dtwT[j] = np.asarray(inp["dt_proj_w"][j, d], np.float32).T
        dtbn[j] = _fold(-np.asarray(inp["dt_proj_b"][j, d], np.float32))
        a = np.exp(np.asarray(inp["A_log"][j, d], np.float32))  # (1536, 16)
        assert np.allclose(a, a[0:1, :], rtol=1e-5), "A_log not d-constant"
        apos[j] = np.tile(a[0], (P, 1))
        dsk[j] = _fold(inp["D_skip"][j, d])
        owT[j] = np.asarray(inp["out_proj_w"][j, d], np.float32).T

    sel = np.zeros((P, 2), np.float32)
    sel[:, d] = 1.0
    return {
        "x0T": f32(np.asarray(inp["input_data"][b], np.float32).T),
        "lnw": f32(lnw), "lnb": f32(lnb),
        "inwT": bf(inwT), "cb": f32(cb),
        "xpwT": bf(xpwT), "dtwT": bf(dtwT), "dtbn": f32(dtbn),
        "apos": f32(apos), "dsk": f32(dsk), "owT": bf(owT),
        "fw": f32(_fold(inp["norm_f_w"])), "fb": f32(_fold(inp["norm_f_b"])),
        "sel": sel,
    }


def kernel(**inputs):
    if "nc" not in _CACHE:
        _CACHE["nc"] = _build()
    nc = _CACHE["nc"]
    in_maps = [_prep_core_inputs(inputs, c % 4, c // 4) for c in range(8)]
    res = run_bass_kernel_spmd(nc, in_maps, core_ids=list(range(8)))
    out = np.empty((B, L, D), np.float32)
    for b in range(B):
        out[b] = res.results[b]["o_fm"].T
    return out


# revision 16
# speedup vs baseline: 14.8522x; 14.8522x over previous
"""Bidirectional Mamba (4 layers x 2 dirs, B=4, L=1024, d_model=768,
d_inner=1536, d_state=16) on 8 TRN2 NeuronCores.

Sharding: core c handles (batch b = c%4, direction d = c//4). Each core runs
the full 4-layer stack for its (b, dir) stream in feature-major layout
[channels-on-partitions, tokens-on-free]. The per-layer h = out_fwd + out_rev
exchange is a pair AllReduce over cores {c, c+4}. The kernel program is
identical on all cores (SPMD); the direction flip is handled by per-core 0/1
select masks applied to (h, flip(h)), and all other per-dir differences are
input data.

Key mappings:
 - depthwise causal conv folded into the in_proj matmul on PE:
   host precomputes W_k[d,e] = conv_w[d,k] * in_w_x[d,e]; the kernel runs 4
   accumulated matmuls against token-shifted views of a zero-left-padded hn.
 - selective scan: p = sigmoid(-(dt_raw+dtb)) so logp = -delta; per state i,
   dA_i = exp(apos_i * logp) on ACT (apos = exp(A_log) per-partition scale),
   dB_i = w * (-B_i) with w = logp*u (so w = -delta*u), then
   tensor_tensor_scan on DVE, prod = s*C_i, accumulated over i.
 - LN in feature-major via ones-matmul column sums on PE.
"""
import os
import numpy as np
import ml_dtypes
from contextlib import ExitStack

import concourse.bacc as bacc
import concourse.tile as tile
import concourse.mybir as mybir
from concourse.bass_utils import run_bass_kernel_spmd

F32 = mybir.dt.float32
BF16 = mybir.dt.bfloat16
MUL = mybir.AluOpType.mult
ADD = mybir.AluOpType.add
SUB = mybir.AluOpType.subtract
AF = mybir.ActivationFunctionType

P = 128
B, L, D, DI, DS, DR, DC, NL = 4, 1024, 768, 1536, 16, 48, 4, 4
EPS = 1e-5
KF = D // P        # 6  k-folds of d_model
MF = DI // P       # 12 folds of d_inner
NT = L // 512      # 2  N-tiles per token row
CH = 2             # folds per scan chunk
NCH = MF // CH     # 6 chunks
MARG = DC - 1      # 3 left-pad columns for causal conv

_CACHE = {}


def _build():
    KNL = int(os.environ.get("K_NL", NL))
    KPH = int(os.environ.get("K_PH", 9))
    nc = bacc.Bacc("TRN2", target_bir_lowering=False)

    # ---------------- dram I/O ----------------
    x0T_d = nc.dram_tensor("x0T", [D, L], F32, kind="ExternalInput")
    lnw_d = nc.dram_tensor("lnw", [NL, P, KF], F32, kind="ExternalInput")
    lnb_d = nc.dram_tensor("lnb", [NL, P, KF], F32, kind="ExternalInput")
    inwT_d = nc.dram_tensor("inwT", [NL, D, 5 * DI], BF16, kind="ExternalInput")
    cb_d = nc.dram_tensor("cb", [NL, P, MF], F32, kind="ExternalInput")
    xpwT_d = nc.dram_tensor("xpwT", [NL, DI, 96], BF16, kind="ExternalInput")
    dtwT_d = nc.dram_tensor("dtwT", [NL, DR, DI], BF16, kind="ExternalInput")
    dtbn_d = nc.dram_tensor("dtbn", [NL, P, MF], F32, kind="ExternalInput")
    apos_d = nc.dram_tensor("apos", [NL, P, DS], F32, kind="ExternalInput")
    dsk_d = nc.dram_tensor("dsk", [NL, P, MF], F32, kind="ExternalInput")
    owT_d = nc.dram_tensor("owT", [NL, DI, D], BF16, kind="ExternalInput")
    fw_d = nc.dram_tensor("fw", [P, KF], F32, kind="ExternalInput")
    fb_d = nc.dram_tensor("fb", [P, KF], F32, kind="ExternalInput")
    sel_d = nc.dram_tensor("sel", [P, 2], F32, kind="ExternalInput")
    o_d = nc.dram_tensor("o_fm", [D, L], F32, kind="ExternalOutput")

    z_dram = nc.dram_tensor("z_sp", [MF, P, L], BF16)
    u_dram = nc.dram_tensor("u_sp", [MF, P, L], BF16)
    y_dram = nc.dram_tensor("y_sp", [MF, P, L], BF16)
    g_dram = nc.dram_tensor("g_sp", [MF, P, L], BF16)
    bc_dram = nc.dram_tensor("bc_sp", [2 * DS, L], BF16)
    cc_in = [nc.dram_tensor(f"cc_in{j}", [D, L], F32) for j in range(NL)]
    cc_out = [nc.dram_tensor(f"cc_out{j}", [D, L], F32) for j in range(NL)]

    with tile.TileContext(nc) as tc, ExitStack() as ctx:
        pers = ctx.enter_context(tc.tile_pool(name="pers", bufs=1))
        vpool = ctx.enter_context(tc.tile_pool(name="vpool", bufs=2))
        big = ctx.enter_context(tc.tile_pool(name="big", bufs=1))
        stg = ctx.enter_context(tc.tile_pool(name="stg", bufs=5))
        stg32 = ctx.enter_context(tc.tile_pool(name="stg32", bufs=4))
        wbig = ctx.enter_context(tc.tile_pool(name="wbig", bufs=3))
        prm = ctx.enter_context(tc.tile_pool(name="prm", bufs=2))
        dApool = ctx.enter_context(tc.tile_pool(name="dApool", bufs=2))
        dBpool = ctx.enter_context(tc.tile_pool(name="dBpool", bufs=3))
        spool = ctx.enter_context(tc.tile_pool(name="spool", bufs=2))
        accp = ctx.enter_context(tc.tile_pool(name="accp", bufs=3))
        bcp = ctx.enter_context(tc.tile_pool(name="bcp", bufs=4))
        mm = ctx.enter_context(tc.tile_pool(name="mm", bufs=8, space="PSUM"))

        # persistent across layers
        h32 = pers.tile([P, KF, L], F32, name="h32")
        res16 = vpool.tile([P, KF, L], BF16, name="res16", tag="resp")
        ones16 = pers.tile([P, 1], BF16, name="ones16")
        sel_sb = pers.tile([P, 2], F32, name="sel_sb")
        nc.vector.memset(res16[:], 0.0)
        nc.vector.memset(ones16[:], 1.0)
        nc.sync.dma_start(sel_sb[:], sel_d[:])
        nc.sync.dma_start(h32[:], x0T_d[:].rearrange("(f p) l -> p f l", p=P))

        def ln_feature_major(vin, wcol, bcol, hn_out, out_off, fp32_rows):
            """LN over the 768 partition-channels of vin [P,KF,L] (bf16).
            hn_out[:, f, out_off:out_off+L] gets normalized*w+b output."""
            ps_s = [mm.tile([P, 512], F32, name="lnps", tag="ps") for _ in range(2 * NT)]
            for f in range(KF):
                sq = stg.tile([P, L], BF16, name="stg_a", tag="st16")
                nc.scalar.activation(sq[:], vin[:, f, :], AF.Square)
                for n in range(NT):
                    nc.tensor.matmul(
                        ps_s[n][0:1, :], ones16[:], vin[:, f, n * 512:(n + 1) * 512],
                        start=(f == 0), stop=(f == KF - 1))
                    nc.tensor.matmul(
                        ps_s[NT + n][0:1, :], ones16[:], sq[:, n * 512:(n + 1) * 512],
                        start=(f == 0), stop=(f == KF - 1))
            mu_r = stg32.tile([1, L], F32, name="mu_r", tag="st32")
            for n in range(NT):
                nc.vector.tensor_scalar(
                    out=mu_r[:, n * 512:(n + 1) * 512], in0=ps_s[n][0:1, :],
                    scalar1=1.0 / D, scalar2=None, op0=MUL)
            mu2_r = stg32.tile([1, L], F32, name="mu2_r", tag="st32")
            nc.vector.tensor_tensor(out=mu2_r[:], in0=mu_r[:], in1=mu_r[:], op=MUL)
            var_r = stg32.tile([1, L], F32, name="var_r", tag="st32")
            for n in range(NT):
                nc.vector.scalar_tensor_tensor(
                    out=var_r[:, n * 512:(n + 1) * 512], in0=ps_s[NT + n][0:1, :],
                    scalar=1.0 / D, in1=mu2_r[:, n * 512:(n + 1) * 512],
                    op0=MUL, op1=SUB)
            eps_r = stg32.tile([1, L], F32, name="eps_r", tag="st32")
            nc.vector.memset(eps_r[:], EPS)
            sd_r = stg32.tile([1, L], F32, name="sd_r", tag="st32")
            nc.scalar.activation(sd_r[:], var_r[:], AF.Sqrt, bias=eps_r[:, 0:1])
            rstd_r = stg32.tile([1, L], F32, name="rstd_r", tag="st32")
            nc.vector.reciprocal(rstd_r[:], sd_r[:])
            dt_b = F32 if fp32_rows else BF16
            mu_b = (stg32 if fp32_rows else bcp).tile([P, L], dt_b, name="mu_b", tag="st32" if fp32_rows else "nbcb")
            rstd_b = (stg32 if fp32_rows else bcp).tile([P, L], dt_b, name="rstd_b", tag="st32" if fp32_rows else "nbcb")
            if fp32_rows:
                nc.gpsimd.partition_broadcast(mu_b[:], mu_r[:])
                nc.gpsimd.partition_broadcast(rstd_b[:], rstd_r[:])
            else:
                mu16_r = stg.tile([1, L], BF16, name="mu16_r", tag="st16")
                rstd16_r = stg.tile([1, L], BF16, name="rstd16_r", tag="st16")
                nc.vector.tensor_copy(mu16_r[:], mu_r[:])
                nc.vector.tensor_copy(rstd16_r[:], rstd_r[:])
                nc.gpsimd.partition_broadcast(mu_b[:], mu16_r[:])
                nc.gpsimd.partition_broadcast(rstd_b[:], rstd16_r[:])
            for f in range(KF):
                st1 = stg.tile([P, L], dt_b, name="stg_b", tag="st16")
                nc.vector.tensor_tensor(out=st1[:], in0=vin[:, f, :], in1=mu_b[:], op=SUB)
                st2 = stg.tile([P, L], dt_b, name="stg_c", tag="st16")
                nc.vector.tensor_tensor(out=st2[:], in0=st1[:], in1=rstd_b[:], op=MUL)
                nc.vector.scalar_tensor_tensor(
                    out=hn_out[:, f, out_off:out_off + L], in0=st2[:],
                    scalar=wcol[:, f:f + 1],
                    in1=bcol[:, f:f + 1].to_broadcast([P, L]),
                    op0=MUL, op1=ADD)

        for j in range(KNL):
            # ---- per-layer params ----
            lnw = prm.tile([P, KF], F32, name="lnw")
            lnb = prm.tile([P, KF], F32, name="lnb")
            cbt = prm.tile([P, MF], F32, name="cbt")
            dtbn = prm.tile([P, MF], F32, name="dtbn")
            apos = prm.tile([P, DS], F32, name="apos")
            dsk = prm.tile([P, MF], F32, name="dsk")
            nc.sync.dma_start(lnw[:], lnw_d[j])
            nc.sync.dma_start(lnb[:], lnb_d[j])
            nc.sync.dma_start(cbt[:], cb_d[j])
            nc.sync.dma_start(dtbn[:], dtbn_d[j])
            nc.sync.dma_start(apos[:], apos_d[j])
            nc.sync.dma_start(dsk[:], dsk_d[j])

            # ---- v = sel0*h + sel1*flip(h) + res ; res' = h + flip(h) + 2res
            v16a = vpool.tile([P, KF, L], BF16, name="v16", tag="vp")
            nc.vector.scalar_tensor_tensor(
                out=v16a[:], in0=h32[:], scalar=sel_sb[:, 0:1], in1=res16[:],
                op0=MUL, op1=ADD)
            v16 = vpool.tile([P, KF, L], BF16, name="v16b", tag="vp")
            nc.vector.scalar_tensor_tensor(
                out=v16[:], in0=h32[:, :, ::-1], scalar=sel_sb[:, 1:2], in1=v16a[:],
                op0=MUL, op1=ADD)
            tmp16 = vpool.tile([P, KF, L], BF16, name="tmp16", tag="vp")
            nc.vector.tensor_tensor(out=tmp16[:], in0=h32[:], in1=h32[:, :, ::-1], op=ADD)
            res_new = vpool.tile([P, KF, L], BF16, name="res_new", tag="resp")
            nc.vector.scalar_tensor_tensor(
                out=res_new[:], in0=res16[:], scalar=2.0, in1=tmp16[:], op0=MUL, op1=ADD)
            res16 = res_new

            # ---- LN -> hn (left-padded by MARG zero cols) ----
            hn16 = big.tile([P, KF, MARG + L], BF16, name="hn16", tag="bigA")
            nc.vector.memset(hn16[:, :, 0:MARG], 0.0)
            ln_feature_major(v16, lnw, lnb, hn16, MARG, fp32_rows=False)

            if KPH < 1:
                continue
            # ---- in_proj x-half with folded conv -> silu -> u ----
            for m in range(MF):
                ps = [mm.tile([P, 512], F32, name="ps_ip", tag="ps") for _ in range(NT)]
                for k in range(KF):
                    wk = wbig.tile([P, 4 * P], BF16, name="wk_ip", tag="w")
                    # lhsT slab: rows k-fold, cols = 4 taps x this m tile
                    nc.sync.dma_start(
                        wk[:].rearrange("p (t q) -> p t q", t=4),
                        inwT_d[j, k * P:(k + 1) * P, :]
                        .rearrange("p (t di) -> p t di", t=5)[:, 0:4, m * P:(m + 1) * P])
                    for tap in range(DC):
                        for n in range(NT):
                            nc.tensor.matmul(
                                ps[n], wk[:, tap * P:(tap + 1) * P],
                                hn16[:, k, tap + n * 512: tap + n * 512 + 512],
                                start=(k == 0 and tap == 0),
                                stop=(k == KF - 1 and tap == DC - 1))
                u_st = stg.tile([P, L], BF16, name="stg_u", tag="st16")
                for n in range(NT):
                    nc.scalar.activation(
                        u_st[:, n * 512:(n + 1) * 512], ps[n], AF.Silu,
                        bias=cbt[:, m:m + 1])
                nc.sync.dma_start(u_dram[m], u_st[:])

            if KPH < 2:
                continue
            # ---- in_proj z-half -> z_dram ----
            for m in range(MF):
                ps = [mm.tile([P, 512], F32, name="ps_ip", tag="ps") for _ in range(NT)]
                for k in range(KF):
                    wz = wbig.tile([P, P], BF16, name="wz_ip", tag="w")
                    nc.sync.dma_start(
                        wz[:], inwT_d[j, k * P:(k + 1) * P,
                                      4 * DI + m * P:4 * DI + (m + 1) * P])
                    for n in range(NT):
                        nc.tensor.matmul(
                            ps[n], wz[:],
                            hn16[:, k, MARG + n * 512: MARG + n * 512 + 512],
                            start=(k == 0), stop=(k == KF - 1))
                z_st = stg.tile([P, L], BF16, name="stg_z", tag="st16")
                for n in range(NT):
                    eng = nc.vector if n == 0 else nc.scalar
                    if n == 0:
                        nc.vector.tensor_copy(z_st[:, 0:512], ps[0])
                    else:
                        nc.scalar.copy(z_st[:, 512:1024], ps[1])
                nc.sync.dma_start(z_dram[m], z_st[:])

            if KPH < 3:
                continue
            # ---- x_proj ----
            ps_xd = [mm.tile([P, 512], F32, name="ps_xd", tag="ps") for _ in range(NT)]
            for k in range(MF):
                xw = wbig.tile([P, 96], BF16, name="xw_xp", tag="w")
                nc.sync.dma_start(xw[:], xpwT_d[j, k * P:(k + 1) * P, :])
                u_rd = stg.tile([P, L], BF16, name="stg_u2", tag="st16")
                nc.sync.dma_start(u_rd[:], u_dram[k])
                for n in range(NT):
                    nc.tensor.matmul(
                        ps_xd[n][0:96, :], xw[:], u_rd[:, n * 512:(n + 1) * 512],
                        start=(k == 0), stop=(k == MF - 1))
            xd16 = prm.tile([96, L], BF16, name="xd16")
            for n in range(NT):
                sl = slice(n * 512, (n + 1) * 512)
                nc.vector.tensor_copy(xd16[0:DR, sl], ps_xd[n][0:DR, :])
                nc.scalar.copy(xd16[64:96, sl], ps_xd[n][64:96, :])
            nc.sync.dma_start(bc_dram[:], xd16[64:96, :])

            if KPH < 4:
                continue
            # ---- dt_proj ----
            logp16 = big.tile([P, MF, L], BF16, name="logp16", tag="bigB")
            for m in range(MF):
                ps = [mm.tile([P, 512], F32, name="ps_dt", tag="ps") for _ in range(NT)]
                dw = wbig.tile([DR, P], BF16, name="dw_dt", tag="w")
                nc.sync.dma_start(dw[:], dtwT_d[j, :, m * P:(m + 1) * P])
                for n in range(NT):
                    nc.tensor.matmul(ps[n], dw[:], xd16[0:DR, n * 512:(n + 1) * 512],
                                     start=True, stop=True)
                for n in range(NT):
                    p_st = stg.tile([P, 512], BF16, name="stg_p", tag="st16")
                    nc.scalar.activation(p_st[:], ps[n], AF.Sigmoid,
                                         bias=dtbn[:, m:m + 1], scale=-1.0)
                    nc.scalar.activation(logp16[:, m, n * 512:(n + 1) * 512],
                                         p_st[:], AF.Ln)

            if KPH < 5:
                continue
            # ---- w = logp*u ----
            w16 = big.tile([P, MF, L], BF16, name="w16", tag="bigA")
            for k in range(MF):
                u_rd = stg.tile([P, L], BF16, name="stg_u3", tag="st16")
                nc.sync.dma_start(u_rd[:], u_dram[k])
                nc.vector.scalar_tensor_tensor(
                    out=w16[:, k, :], in0=logp16[:, k, :], scalar=-1.0,
                    in1=u_rd[:], op0=MUL, op1=MUL)
            # poison t=0 so fold-chained scans reset state at fold boundaries
            nc.vector.memset(logp16[:, :, 0:1], -30000.0)

            if KPH < 6:
                continue
            # ---- scan ----
            for ch in range(NCH):
                fs = slice(ch * CH, (ch + 1) * CH)
                accE = None
                accO = None
                for i in range(DS):
                    nb = bcp.tile([P, L], BF16, name="nb_bc", tag="nbcb")
                    nc.sync.dma_start(nb[:], bc_dram[i:i + 1, :].to_broadcast([P, L]))
                    cb_i = bcp.tile([P, L], BF16, name="cb_bc", tag="nbcb")
                    nc.sync.dma_start(cb_i[:], bc_dram[DS + i:DS + i + 1, :]
                                      .to_broadcast([P, L]))
                    dA = dApool.tile([P, CH, L], BF16, name="dA")
                    nc.scalar.activation(
                        dA[:].rearrange("p a b -> p (a b)"),
                        logp16[:, fs, :].rearrange("p a b -> p (a b)"),
                        AF.Exp, scale=apos[:, i:i + 1])
                    dB = dBpool.tile([P, CH, L], BF16, name="dB", tag="dBp")
                    nc.vector.tensor_tensor(
                        out=dB[:], in0=w16[:, fs, :],
                        in1=nb[:, None, :].to_broadcast([P, CH, L]), op=MUL)
                    s16 = spool.tile([P, CH, L], BF16, name="s16")
                    nc.vector.tensor_tensor_scan(
                        s16[:].rearrange("p a b -> p (a b)"),
                        dA[:].rearrange("p a b -> p (a b)"),
                        dB[:].rearrange("p a b -> p (a b)"),
                        0.0, MUL, ADD)
                    if i == 1:
                        prod = accp.tile([P, CH, L], BF16, name="acc")
                    else:
                        prod = dBpool.tile([P, CH, L], BF16, name="prod", tag="dBp")
                    nc.vector.tensor_tensor(
                        out=prod[:], in0=s16[:],
                        in1=cb_i[:, None, :].to_broadcast([P, CH, L]), op=MUL)
                    if i < 2:
                        if i == 0:
                            # initialize even chain with u*D_skip + prod
                            tgt = accp.tile([P, CH, L], BF16, name="acc")
                            for fo in range(CH):
                                u_rd = stg.tile([P, L], BF16, name="stg_u4", tag="st16")
                                nc.sync.dma_start(u_rd[:], u_dram[ch * CH + fo])
                                nc.vector.scalar_tensor_tensor(
                                    out=tgt[:, fo, :], in0=u_rd[:],
                                    scalar=dsk[:, ch * CH + fo:ch * CH + fo + 1],
                                    in1=prod[:, fo, :], op0=MUL, op1=ADD)
                            accE = tgt
                        else:
                            accO = prod
                    else:
                        src = accE if (i % 2 == 0) else accO
                        tgt = accp.tile([P, CH, L], BF16, name="acc")
                        nc.vector.tensor_tensor(out=tgt[:], in0=src[:], in1=prod[:], op=ADD)
                        if i % 2 == 0:
                            accE = tgt
                        else:
                            accO = tgt
                yroot = spool.tile([P, CH, L], BF16, name="s16")
                nc.vector.tensor_tensor(out=yroot[:], in0=accE[:], in1=accO[:], op=ADD)
                nc.sync.dma_start(
                    y_dram[fs].rearrange("f p t -> p f t"), yroot[:])

            if KPH < 7:
                continue
            # ---- gate ----
            for f in range(MF):
                y_st = stg.tile([P, L], BF16, name="stg_y", tag="st16")
                nc.sync.dma_start(y_st[:], y_dram[f])
                z_st = stg.tile([P, L], BF16, name="stg_z2", tag="st16")
                nc.sync.dma_start(z_st[:], z_dram[f])
                zs = stg.tile([P, L], BF16, name="stg_zs", tag="st16")
                nc.scalar.activation(zs[:], z_st[:], AF.Silu)
                g_st = stg.tile([P, L], BF16, name="stg_g", tag="st16")
                nc.vector.tensor_tensor(out=g_st[:], in0=y_st[:], in1=zs[:], op=MUL)
                nc.sync.dma_start(g_dram[f], g_st[:])

            if KPH < 8:
                continue
            # ---- out_proj ----
            for half in range(2):
                ms = range(half * 3, half * 3 + 3)
                ps_o = {(m, n): mm.tile([P, 512], F32, name="ps_op", tag="ps")
                        for m in ms for n in range(NT)}
                for k in range(MF):
                    ow = wbig.tile([P, D], BF16, name="ow_op", tag="w")
                    nc.sync.dma_start(ow[:], owT_d[j, k * P:(k + 1) * P, :])
                    g_rd = stg.tile([P, L], BF16, name="stg_g2", tag="st16")
                    nc.sync.dma_start(g_rd[:], g_dram[k])
                    for m in ms:
                        for n in range(NT):
                            nc.tensor.matmul(
                                ps_o[(m, n)], ow[:, m * P:(m + 1) * P],
                                g_rd[:, n * 512:(n + 1) * 512],
                                start=(k == 0), stop=(k == MF - 1))
                for m in ms:
                    for n in range(NT):
                        o_ev = stg32.tile([P, 512], F32, name="stg_ev", tag="st32")
                        if (m + n) % 2 == 0:
                            nc.vector.tensor_copy(o_ev[:], ps_o[(m, n)])
                        else:
                            nc.scalar.copy(o_ev[:], ps_o[(m, n)])
                        nc.sync.dma_start(
                            cc_in[j][m * P:(m + 1) * P, n * 512:(n + 1) * 512],
                            o_ev[:])

            nc.gpsimd.collective_compute(
                kind="AllReduce", op=ADD,
                replica_groups=[[0, 4], [1, 5], [2, 6], [3, 7]],
                ins=[cc_in[j][:]], outs=[cc_out[j][:]])
            h_new = pers.tile([P, KF, L], F32, name="h32", tag="h32")
            nc.sync.dma_start(h_new[:], cc_out[j][:].rearrange("(f p) l -> p f l", p=P))
            h32 = h_new

        # ---- final: out = LN(h + res) ----
        vf32 = big.tile([P, KF, L], F32, name="vf32", tag="bigA")
        nc.vector.tensor_tensor(out=vf32[:], in0=h32[:], in1=res16[:], op=ADD)
        vf16 = vpool.tile([P, KF, L], BF16, name="vf16", tag="vp")
        nc.vector.tensor_copy(vf16[:], vf32[:])
        fw = prm.tile([P, KF], F32, name="fw")
        fb = prm.tile([P, KF], F32, name="fb")
        nc.sync.dma_start(fw[:], fw_d[:])
        nc.sync.dma_start(fb[:], fb_d[:])
        # fp32 stats+normalize for the final output
        ps_s = [mm.tile([P, 512], F32, name="lnps2", tag="ps") for _ in range(2 * NT)]
        for f in range(KF):
            sq = stg.tile([P, L], BF16, name="stg_q", tag="st16")
            nc.scalar.activation(sq[:], vf16[:, f, :], AF.Square)
            for n in range(NT):
                nc.tensor.matmul(ps_s[n][0:1, :], ones16[:],
                                 vf16[:, f, n * 512:(n + 1) * 512],
                                 start=(f == 0), stop=(f == KF - 1))
                nc.tensor.matmul(ps_s[NT + n][0:1, :], ones16[:],
                                 sq[:, n * 512:(n + 1) * 512],
                                 start=(f == 0), stop=(f == KF - 1))
        mu_r = stg32.tile([1, L], F32, name="mu_rf", tag="st32")
        for n in range(NT):
            nc.vector.tensor_scalar(out=mu_r[:, n * 512:(n + 1) * 512],
                                    in0=ps_s[n][0:1, :], scalar1=1.0 / D,
                                    scalar2=None, op0=MUL)
        mu2_r = stg32.tile([1, L], F32, name="mu2_rf", tag="st32")
        nc.vector.tensor_tensor(out=mu2_r[:], in0=mu_r[:], in1=mu_r[:], op=MUL)
        var_r = stg32.tile([1, L], F32, name="var_rf", tag="st32")
        for n in range(NT):
            nc.vector.scalar_tensor_tensor(
                out=var_r[:, n * 512:(n + 1) * 512], in0=ps_s[NT + n][0:1, :],
                scalar=1.0 / D, in1=mu2_r[:, n * 512:(n + 1) * 512], op0=MUL, op1=SUB)
        eps_rf = stg32.tile([1, L], F32, name="eps_rf", tag="st32")
        nc.vector.memset(eps_rf[:], EPS)
        sd_rf = stg32.tile([1, L], F32, name="sd_rf", tag="st32")
        nc.scalar.activation(sd_rf[:], var_r[:], AF.Sqrt, bias=eps_rf[:, 0:1])
        rstd_r = stg32.tile([1, L], F32, name="rstd_rf", tag="st32")
        nc.vector.reciprocal(rstd_r[:], sd_rf[:])
        mu_b = stg32.tile([P, L], F32, name="mu_bf", tag="st32")
        rstd_b = stg32.tile([P, L], F32, name="rstd_bf", tag="st32")
        nc.gpsimd.partition_broadcast(mu_b[:], mu_r[:])
        nc.gpsimd.partition_broadcast(rstd_b[:], rstd_r[:])
        for f in range(KF):
            st1 = stg32.tile([P, L], F32, name="stg_f1", tag="st32")
            nc.vector.tensor_tensor(out=st1[:], in0=vf32[:, f, :], in1=mu_b[:], op=SUB)
            st2 = stg32.tile([P, L], F32, name="stg_f2", tag="st32")
            nc.vector.tensor_tensor(out=st2[:], in0=st1[:], in1=rstd_b[:], op=MUL)
            o_st = stg32.tile([P, L], F32, name="stg_f3", tag="st32")
            nc.vector.scalar_tensor_tensor(
                out=o_st[:], in0=st2[:], scalar=fw[:, f:f + 1],
                in1=fb[:, f:f + 1].to_broadcast([P, L]), op0=MUL, op1=ADD)
            nc.sync.dma_start(o_d[f * P:(f + 1) * P, :], o_st[:])

    nc.compile()
    return nc


def _fold(x):
    """[C] -> [P, C/P] fold-major (channel c = fold*128 + p)."""
    x = np.asarray(x, np.float32)
    nf = x.shape[-1] // P
    return np.ascontiguousarray(x.reshape(nf, P).T)


def _prep_core_inputs(inputs, b, d):
    bf = lambda x: np.ascontiguousarray(np.asarray(x)).astype(ml_dtypes.bfloat16)
    f32 = lambda x: np.ascontiguousarray(np.asarray(x, np.float32))
    inp = {k: np.asarray(v) for k, v in inputs.items()}

    inwT = np.empty((NL, D, 5 * DI), np.float32)
    lnw = np.empty((NL, P, KF), np.float32)
    lnb = np.empty((NL, P, KF), np.float32)
    cb = np.empty((NL, P, MF), np.float32)
    xpwT = np.zeros((NL, DI, 96), np.float32)
    dtwT = np.empty((NL, DR, DI), np.float32)
    dtbn = np.empty((NL, P, MF), np.float32)
    apos = np.empty((NL, P, DS), np.float32)
    dsk = np.empty((NL, P, MF), np.float32)
    owT = np.empty((NL, DI, D), np.float32)
    for j in range(NL):
        iw = np.asarray(inp["in_proj_w"][j, d], np.float32)   # (3072, 768)
        cw = np.asarray(inp["conv_w"][j, d], np.float32)      # (1536, 4)
        wx, wz = iw[:DI], iw[DI:]
        parts = [ (wx * cw[:, k:k + 1]).T for k in range(DC) ] + [wz.T]
        inwT[j] = np.concatenate(parts, axis=1)
        lnw[j] = _fold(inp["norm_w"][j, d])
        lnb[j] = _fold(inp["norm_b"][j, d])
        cb[j] = _fold(inp["conv_b"][j, d])
        xpw_t = np.asarray(inp["x_proj_w"][j, d], np.float32).T
        xpwT[j, :, 0:DR] = xpw_t[:, 0:DR]
        xpwT[j, :, 64:80] = xpw_t[:, DR:DR + DS]
        xpwT[j, :, 80:96] = xpw_t[:, DR + DS:80]
        dtwT[j] = np.asarray(inp["dt_proj_w"][j, d], np.float32).T
        dtbn[j] = _fold(-np.asarray(inp["dt_proj_b"][j, d], np.float32))
        a = np.exp(np.asarray(inp["A_log"][j, d], np.float32))  # (1536, 16)
        assert np.allclose(a, a[0:1, :], rtol=1e-5), "A_log not d-constant"
        apos[j] = np.tile(a[0], (P, 1))
        dsk[j] = _fold(inp["D_skip"][j, d])
        owT[j] = np.asarray(inp["out_proj_w"][j, d], np.float32).T

    sel = np.zeros((P, 2), np.float32)
    sel[:, d] = 1.0
    return {
        "x0T": f32(np.asarray(inp["input_data"][b], np.float32).T),
        "lnw": f32(lnw), "lnb": f32(lnb),
        "inwT": bf(inwT), "cb": f32(cb),
        "xpwT": bf(xpwT), "dtwT": bf(dtwT), "dtbn": f32(dtbn),
        "apos": f32(apos), "dsk": f32(dsk), "owT": bf(owT),
        "fw": f32(_fold(inp["norm_f_w"])), "fb": f32(_fold(inp["norm_f_b"])),
        "sel": sel,
    }


def kernel(**inputs):
    if "nc" not in _CACHE:
        _CACHE["nc"] = _build()
    nc = _CACHE["nc"]
    in_maps = [_prep_core_inputs(inputs, c % 4, c // 4) for c in range(8)]
    try:
        res = run_bass_kernel_spmd(nc, in_maps, core_ids=list(range(8)))
    except Exception:
        import time as _time
        _time.sleep(5)
        res = run_bass_kernel_spmd(nc, in_maps, core_ids=list(range(8)))
    out = np.empty((B, L, D), np.float32)
    for b in range(B):
        out[b] = res.results[b]["o_fm"].T
    return out
